# revision 24
# baseline (speedup 1.0000x reference)
"""Bias-augmented attention (AlphaFold-style) on 8 Trainium2 NeuronCores.

Problem: B=1, Q=K=2048, C_IN=256, H=8, CH=32
    q = (q_x @ w_q) / sqrt(CH); k = kv_x @ w_k; v = kv_x @ w_v   (per head)
    a = softmax(q k^T + pair_bias + mask_bias)
    o = (a v) * sigmoid(q_x @ w_g + b_g)
    out = o @ w_o + b_o

Sharding: data-parallel over query rows. Core i handles q rows
[256*i, 256*(i+1)), all 8 heads.

v2 layout (changes vs the 84us baseline, driven by the NTFF trace):
  * pair_bias DMA'd in 32 big transfers (2KB contiguous per partition,
    256KB each) instead of 64 small ones: the Sync engine's ~600ns
    serial DMA_DIRECT2D issue rate was the main-loop bottleneck.
  * gate sigmoids become tanh (same ACT table set as exp) packed into
    128 partitions: sigmoid(z) = 0.5*tanh(z/2)+0.5. One table set for
    the whole kernel; a dummy exp at t=0 preloads it off the critical
    path. Gate matmuls use col-group tile_position to pack 4 heads
    onto the 128 PSUM partitions; two tanh ACTIVATEs cover all heads.
  * w_o, gate, and output-projection path in fp16 (fp32r matmuls ran
    in fp32-HIGH mode at ~410ns vs ~110ns fp16).
  * y exported per head-pair as fp16 (4 DMAs instead of 8 fp32 ones).
  * scores S^T computed transposed with a ones-augmented V (M=33) so
    one accumulating matmul chain gives numerator and denominator;
    mask folds in as exp(mask) row scaling of V-hat; exp runs with a
    -3 bias (cancels on the host); normalization happens on the host.
  * pair folds into scores via fp16 identity-matmul PSUM accumulation;
    A@V alternates even/odd-chunk accumulators in different PSUM banks
    and PE column groups; emission software-pipelines QK ahead of A@V.
"""

import math
import sys

for _p in ("/opt/trn_rl_repo",):
    if _p not in sys.path:
        sys.path.insert(0, _p)

import numpy as np

import concourse.bass as bass
import concourse.mybir as mybir
import concourse.tile as tile
from concourse import bacc
from concourse.bass_utils import run_bass_kernel_spmd

F32 = mybir.dt.float32
F16 = mybir.dt.float16

B, Q, K, C, H, CH = 1, 2048, 2048, 256, 8, 32
NCORES = 8
QS = Q // NCORES  # 256 query rows per core
KC = K // 128  # 16 key chunks of 128


def build_nc():
    nc = bacc.Bacc("TRN2", target_bir_lowering=False, debug=False)

    # ---- DRAM I/O (per-core shard shapes) ----
    # pairT [h][p][kc][q]: per-partition kc-contiguous (8KB runs per head)
    pairT = nc.dram_tensor("pairT", [H, 128, KC, QS], F16, kind="ExternalInput").ap()
    wpack = nc.dram_tensor("wpack", [2, 128, 5 * C], F16, kind="ExternalInput").ap()
    kvxT = nc.dram_tensor("kvxT", [C, K], F16, kind="ExternalInput").ap()
    # w_o as [d=CH, h, c] so each head's slice sits at partition base 0
    wod = nc.dram_tensor("wod", [CH, H, C], F16, kind="ExternalInput").ap()
    # b_g/2 packed [32*(h%4)+d, h//4]
    bgt = nc.dram_tensor("bgt", [128, 2], F32, kind="ExternalInput").ap()
    emx = nc.dram_tensor("emx", [128, KC], F32, kind="ExternalInput").ap()
    ident_d = nc.dram_tensor("ident", [128, 128], F16, kind="ExternalInput").ap()
    y4 = nc.dram_tensor("y4", [4, 128, 2 * 512], F16, kind="ExternalOutput").ap()
    den = nc.dram_tensor("den", [H, QS], F32, kind="ExternalOutput").ap()

    with tile.TileContext(nc) as tc:
        with (
            tc.tile_pool(name="const", bufs=1) as const_pool,
            tc.tile_pool(name="pt", bufs=10) as pt_pool,
            tc.tile_pool(name="exps", bufs=6) as exp_pool,
            tc.tile_pool(name="head", bufs=3) as head_pool,
            tc.tile_pool(name="mm", bufs=2, space="PSUM") as mmsum,
            tc.tile_pool(name="otsum", bufs=1, space="PSUM") as otsum_pool,
            tc.tile_pool(name="yp", bufs=2, space="PSUM") as ypool,
        ):
            # exp bias (-3): keeps E inside fp16 range; cancels on the host
            negc = const_pool.tile([128, 1], F32, tag="negc")
            nc.vector.memset(negc, -3.0)

            # ---- lead-in DMAs, split across both HWDGE rings (Sync+Scalar)
            # plus SWDGE (GpSimd) for small constants: each ring issues one
            # 128-partition DMA per ~650ns, so a single ring serializes the
            # whole lead-in.
            pt_tiles = {}

            def dma_pt(h, j, eng=None):
                key = (h, j)
                pt_tiles[key] = pt_pool.tile([128, 8, QS], F16, tag="pt", name="pt")
                (eng or nc.sync).dma_start(
                    out=pt_tiles[key],
                    in_=pairT[h, :, 8 * j : 8 * j + 8, :],
                )

            # wpack columns: [wq | wk | qxT | wv | wg] so the first-768-col
            # "front" DMA carries everything qT/kT00 needs
            wpk = [
                const_pool.tile([128, 5 * C], F16, tag=f"wpk{s}", name=f"wpk{s}")
                for s in range(2)
            ]
            wq_s = [wpk[s][:, 0:C] for s in range(2)]
            wk_s = [wpk[s][:, C : 2 * C] for s in range(2)]
            qxT_s = [wpk[s][:, 2 * C : 2 * C + QS] for s in range(2)]
            wv_s = [wpk[s][:, 3 * C : 4 * C] for s in range(2)]
            wg_s = [wpk[s][:, 4 * C : 5 * C] for s in range(2)]
            kvxT_s = [
                const_pool.tile([128, K], F16, tag=f"kvxT{st}", name=f"kvxT{st}")
                for st in range(2)
            ]

            def dma_wpk(s, lo, hi, eng):
                eng.dma_start(out=wpk[s][:, lo:hi], in_=wpack[s, :, lo:hi])

            def dma_kvx(st, lo, hi, eng):
                eng.dma_start(
                    out=kvxT_s[st][:, lo:hi],
                    in_=kvxT[128 * st : 128 * (st + 1), lo:hi],
                )

            # Scalar ring: exactly the step-0 criticals, then the dummy exp
            # (nothing may queue after it on ACT or exp0 is FIFO-blocked)
            dma_wpk(1, 0, 3 * C, nc.scalar)
            dma_kvx(1, 0, 512, nc.scalar)
            dma_pt(1, 0, nc.scalar)
            # dummy exp: forces the exp_and_others ACT table load early
            scr = const_pool.tile([128, 1], F32, tag="scr")
            nc.scalar.activation(
                out=scr, in_=negc, func=mybir.ActivationFunctionType.Exp, bias=negc
            )

            # Sync ring, deadline order
            ident_t = const_pool.tile([128, 128], F16, tag="ident")
            nc.sync.dma_start(out=ident_t, in_=ident_d)
            dma_wpk(0, 0, 3 * C, nc.sync)
            dma_kvx(0, 0, 512, nc.sync)
            dma_pt(0, 0)
            dma_wpk(0, 3 * C, 5 * C, nc.sync)
            dma_wpk(1, 3 * C, 5 * C, nc.sync)
            dma_kvx(0, 512, 2048, nc.sync)
            dma_kvx(1, 512, 2048, nc.sync)
            dma_pt(0, 1)
            dma_pt(1, 1)
            dma_pt(2, 0)
            dma_pt(3, 0)

            # SWDGE for the small constants
            em = const_pool.tile([128, KC], F32, tag="em")
            nc.gpsimd.dma_start(out=em, in_=emx)
            bgt_sb = const_pool.tile([128, 2], F32, tag="bgt")
            nc.gpsimd.dma_start(out=bgt_sb, in_=bgt)
            wo_sb = const_pool.tile([CH, H, C], F16, tag="wo")
            nc.gpsimd.dma_start(out=wo_sb, in_=wod)
            wo_h = [wo_sb[:, h, :] for h in range(H)]

            # remaining pair transfers issued 1/step from the streaming loop
            dma_deferred = []
            for h2, j in ((2, 1), (4, 0), (4, 1), (6, 0), (6, 1)):
                dma_deferred.append((h2, j))
                dma_deferred.append((h2 + 1, j))

            # ---- projections ----
            kT = [[None] * (K // 512) for _ in range(2)]
            qT = [None, None]
            vhat = [None] * KC

            def emit_kT(t, n):
                kt_nt = const_pool.tile([128, 512], F16, tag=f"kT{t}_{n}")
                ps = ypool.tile([128, 512], F32, tag="yp", name="ps")
                for srt in range(2):
                    nc.tensor.matmul(
                        ps,
                        wk_s[srt][:, 128 * t : 128 * (t + 1)],
                        kvxT_s[srt][:, 512 * n : 512 * (n + 1)],
                        start=(srt == 0),
                        stop=(srt == 1),
                    )
                nc.vector.tensor_copy(kt_nt, ps)
                kT[t][n] = kt_nt

            def emit_qT(t):
                qT_t = const_pool.tile([128, QS], F16, tag=f"qT{t}")
                ps = ypool.tile([128, 512], F32, tag="yp", name="ps")[:, 0:QS]
                for srt in range(2):
                    nc.tensor.matmul(
                        ps,
                        wq_s[srt][:, 128 * t : 128 * (t + 1)],
                        qxT_s[srt],
                        start=(srt == 0),
                        stop=(srt == 1),
                    )
                nc.vector.tensor_copy(qT_t, ps)
                qT[t] = qT_t

            def emit_vhat(c):
                # vhat[c][p, h, 0:32] = V[128c+p, 32h+d] * exp(mask)[128c+p]
                # vhat[c][p, h, 32]   = exp(mask)[128c+p]
                vh = const_pool.tile([128, H, CH + 1], F16, tag=f"vhat{c}")
                ps = ypool.tile([128, 512], F32, tag="yp", name="ps")[:, 0:C]
                for srt in range(2):
                    nc.tensor.matmul(
                        ps,
                        kvxT_s[srt][:, 128 * c : 128 * (c + 1)],
                        wv_s[srt],
                        start=(srt == 0),
                        stop=(srt == 1),
                    )
                emc = em[:, c : c + 1]
                nc.vector.tensor_scalar_mul(
                    vh[:, :, 0:CH], ps.rearrange("p (h d) -> p h d", h=H), emc
                )
                nc.vector.tensor_copy(vh[:, :, CH : CH + 1], emc.broadcast_to((128, H, 1)))
                vhat[c] = vh

            # ---- gates: g = sigmoid(qx@wg + b_g) = 0.5*tanh((qx@wg)/2 + b_g/2)+0.5
            # (tanh shares the exp ACT table set: no table switch mid-stream).
            # Heads packed on partitions: head h=(4t+s) at rows 32s via
            # col-group tile_position, psum col block 512t so consecutive
            # chains drain alternating banks.
            gate_state = {}
            gth = const_pool.tile([128, 2, QS], F16, tag="gth")
            gta = const_pool.tile([128, 2, QS], F16, tag="gta")
            gT = [None] * H

            def emit_gate_mm(shalf):
                if "gps0" not in gate_state:
                    gate_state["gps0"] = ypool.tile([128, 512], F32, tag="yp", name="g0")
                    gate_state["gps1"] = ypool.tile([128, 512], F32, tag="yp", name="g1")
                for s in (2 * shalf, 2 * shalf + 1):
                    for t in range(2):
                        gps = gate_state[f"gps{t}"]
                        h = 4 * t + s
                        for srt in range(2):
                            nc.tensor.matmul(
                                gps[32 * s : 32 * s + 32, 0:QS],
                                wg_s[srt][:, CH * h : CH * (h + 1)],
                                qxT_s[srt],
                                start=(srt == 0),
                                stop=(srt == 1),
                                tile_position=(0, 32 * s),
                                skip_group_check=True,
                            )

            def emit_gate_act():
                for t in range(2):
                    nc.scalar.activation(
                        out=gth[:, t, :],
                        in_=gate_state[f"gps{t}"][:, 0:QS],
                        func=mybir.ActivationFunctionType.Tanh,
                        scale=0.5,
                        bias=bgt_sb[:, t : t + 1],
                    )
                nc.vector.tensor_scalar(
                    gta, gth, 0.5, 0.5, mybir.AluOpType.mult, mybir.AluOpType.add
                )
                for h in range(H):
                    t, s = h // 4, h % 4
                    g_t = const_pool.tile([CH, QS], F16, tag=f"gT{h}")
                    nc.vector.tensor_copy(g_t, gta[32 * s : 32 * s + 32, t, :])
                    gT[h] = g_t

            emit_qT(0)
            emit_kT(0, 0)

            # per-step deferred work; inner lists are popped one list per step
            deferred = [
                [],
                [("vhat", 0), ("vhat", 1), ("kT", (0, 1))],
                [("vhat", 2), ("vhat", 3)],
                [("kT", (0, 2)), ("vhat", 4), ("vhat", 5)],
                [("kT", (0, 3)), ("vhat", 6), ("vhat", 7)],
                [("kT", (1, 0)), ("vhat", 8), ("vhat", 9)],
                [("kT", (1, 1)), ("vhat", 10), ("vhat", 11)],
                [("kT", (1, 2)), ("vhat", 12), ("vhat", 13)],
                [("kT", (1, 3)), ("vhat", 14), ("vhat", 15)],
                [("qT", None), ("gmm", 0), ("gmm", 1)],
                [("gact", None)],
            ]

            # denominators for all heads, exported once at the end
            den_sb = const_pool.tile([CH + 1, H * QS], F32, tag="den")

            # ---- streaming attention, software-pipelined ----
            # Steps iterate over head PAIRS x chunk-pairs: the two heads of
            # a pair live on adjacent kT/qT row-strips, so their QK matmuls
            # run concurrently on different PE row-groups AND drain into
            # different PSUM banks (same-bank concurrent drains are fatal).
            steps = [(t, p, cg) for t in range(2) for p in range(2) for cg in range(KC // 2)]
            tail_queue = []
            ot_by_pair = {}
            pair_state = {}

            def emit_qk(i):
                t, p, cg = steps[i]
                hA, hB = 4 * t + 2 * p, 4 * t + 2 * p + 1
                c0, c1 = 2 * cg, 2 * cg + 1
                j, m = cg // 4, cg % 4
                ptA = pt_tiles[(hA, j)][:, 2 * m : 2 * m + 2, :]
                ptB = pt_tiles[(hB, j)][:, 2 * m : 2 * m + 2, :]
                # sp halves by head: [hA-c0 | hA-c1 | hB-c0 | hB-c1]
                sp = mmsum.tile([128, 4 * QS], F32, tag="sp", name="sp")
                # issue order alternates banks: hA-c0 (a), hB-c0 (b), hA-c1
                # (a), hB-c1 (b) -> concurrent row-strip pairs never share a
                # draining bank
                for q, (hh, cc) in enumerate(
                    [(2 * p, c0), (2 * p + 1, c0), (2 * p, c1), (2 * p + 1, c1)]
                ):
                    quarter = [0, 2, 1, 3][q]
                    nc.tensor.matmul(
                        sp[:, QS * quarter : QS * (quarter + 1)],
                        kT[t][cc // 4][32 * hh : 32 * hh + 32, 128 * (cc % 4) : 128 * (cc % 4 + 1)],
                        qT[t][32 * hh : 32 * hh + 32, :],
                        start=(q < 2),
                        stop=True,
                        tile_position=(32 * hh, 0),
                        skip_group_check=True,
                    )
                # S^T += pair^T via fp16 identity-matmul accumulate on the PE
                # (half A -> bank a, half B -> bank b)
                for half, pt_half in ((0, ptA), (1, ptB)):
                    nc.tensor.matmul(
                        sp[:, 512 * half : 512 * (half + 1)],
                        ident_t,
                        pt_half.rearrange("p a q -> p (a q)"),
                        start=False,
                        stop=True,
                        skip_group_check=True,
                    )
                e_t = exp_pool.tile([128, 4 * QS], F16, tag="E", name="E")
                nc.scalar.activation(
                    out=e_t, in_=sp, func=mybir.ActivationFunctionType.Exp, bias=negc
                )
                return e_t

            def emit_av(i, e_t):
                t, p, cg = steps[i]
                hA, hB = 4 * t + 2 * p, 4 * t + 2 * p + 1
                c0, c1 = 2 * cg, 2 * cg + 1
                if cg == 0:
                    # one even + one odd accumulator per pair, two heads side
                    # by side: even chunks hit PE column-group 0, odd chunks
                    # column-group 2, in different PSUM banks
                    ot_by_pair[(t, p)] = (
                        otsum_pool.tile([CH + 1, 2 * QS], F32, tag="ote", name="ote"),
                        otsum_pool.tile([97, 2 * QS], F32, tag="oto", name="oto"),
                    )
                ote, oto = ot_by_pair[(t, p)]
                for hh, cc, quarter in (
                    (0, c0, 0),
                    (0, c1, 1),
                    (1, c0, 2),
                    (1, c1, 3),
                ):
                    out, row = (ote, 0) if cc % 2 == 0 else (oto, 64)
                    nc.tensor.matmul(
                        out[row : row + CH + 1, QS * hh : QS * (hh + 1)],
                        vhat[cc][:, (hA, hB)[hh], :],
                        e_t[:, QS * quarter : QS * (quarter + 1)],
                        start=(cg == 0 and hh == 0),
                        stop=(cg == KC // 2 - 1),
                        tile_position=(0, row),
                        skip_group_check=True,
                    )
                if cg == KC // 2 - 1:
                    tail_queue.append(("merge", (t, p)))
                    tail_queue.append(("proj", (t, p, 0)))
                    tail_queue.append(("proj", (t, p, 1)))

            def emit_tail(stage):
                kind, arg = stage
                if kind == "merge":
                    t, p = arg
                    ote, oto = ot_by_pair[(t, p)]
                    # merge even/odd accumulators for both heads at once; the
                    # add reads PSUM at base 64 plus SBUF at base 0 (legal:
                    # only SB+SB bases must match; max one PSUM input)
                    ots = head_pool.tile([CH + 1, 2 * QS], F32, tag="ots", name="ots")
                    nc.vector.tensor_copy(ots, ote)
                    otf = head_pool.tile([CH + 1, 2 * QS], F32, tag="otf", name="otf")
                    nc.vector.tensor_add(otf, oto[64 : 64 + CH + 1, :], ots)
                    hA = 4 * t + 2 * p
                    nc.vector.tensor_copy(
                        den_sb[CH : CH + 1, QS * hA : QS * (hA + 2)],
                        otf[CH : CH + 1, :],
                    )
                    pair_state[(t, p)] = otf
                else:
                    t, p, hh = arg
                    h = 4 * t + 2 * p + hh
                    otf = pair_state[(t, p)]
                    if hh == 0:
                        pair_state[(t, p, "ysb")] = head_pool.tile(
                            [128, 2, 512], F16, tag="ysb", name="ysb"
                        )
                    ysb = pair_state[(t, p, "ysb")]
                    gom = head_pool.tile([CH, QS], F16, tag="gom", name="gom")
                    with nc.allow_low_precision(reason="fp16 gate/proj path"):
                        nc.vector.tensor_mul(
                            gom, otf[0:CH, QS * hh : QS * (hh + 1)], gT[h]
                        )
                    y_ps = ypool.tile([128, 512], F32, tag="yp", name="yps")
                    for qc in range(QS // 128):
                        nc.tensor.matmul(
                            y_ps[:, 256 * qc : 256 * (qc + 1)],
                            gom[:, 128 * qc : 128 * (qc + 1)],
                            wo_h[h],
                            start=(qc == 0),
                            stop=True,
                            skip_group_check=True,
                        )
                    nc.vector.tensor_copy(ysb[:, hh, :], y_ps)
                    if hh == 1:
                        pr = 2 * t + p
                        nc.sync.dma_start(
                            out=y4[pr], in_=ysb.rearrange("p a c -> p (a c)")
                        )

            pending = []
            for i in range(len(steps)):
                e_t = emit_qk(i)
                pending.append((i, e_t))
                if len(pending) > 2:
                    emit_av(*pending.pop(0))
                if deferred:
                    for kind, arg in deferred.pop(0):
                        if kind == "vhat":
                            emit_vhat(arg)
                        elif kind == "kT":
                            emit_kT(*arg)
                        elif kind == "gmm":
                            emit_gate_mm(arg)
                        elif kind == "gact":
                            emit_gate_act()
                        else:
                            emit_qT(1)
                if dma_deferred:
                    dma_pt(*dma_deferred.pop(0))
                if tail_queue:
                    emit_tail(tail_queue.pop(0))
            while pending:
                emit_av(*pending.pop(0))
                if tail_queue:
                    emit_tail(tail_queue.pop(0))
            while tail_queue:
                emit_tail(tail_queue.pop(0))

            # ---- export denominators ----
            nc.sync.dma_start(
                out=den.rearrange("h q -> (h q)"), in_=den_sb[CH : CH + 1, :]
            )

    nc.compile()
    return nc


_NC_CACHE = None


def get_nc():
    global _NC_CACHE
    if _NC_CACHE is None:
        _NC_CACHE = build_nc()
    return _NC_CACHE


def make_in_maps(q_x, kv_x, pair_bias, mask_bias, w_q, w_k, w_v, w_g, b_g, w_o):
    f = np.float32
    q_x = np.asarray(q_x, f)
    kv_x = np.asarray(kv_x, f)
    pair_bias = np.asarray(pair_bias, f)
    mask_bias = np.asarray(mask_bias, f)
    wq16 = (np.asarray(w_q, f) / math.sqrt(CH)).astype(np.float16)
    # b_g/2 packed [32*(h%4)+d, h//4] to match the gate partition layout
    bg2 = (np.asarray(b_g, f) / 2.0).reshape(2, 4, CH).transpose(1, 2, 0).reshape(128, 2)
    shared = {
        "kvxT": np.ascontiguousarray(kv_x[0].T.astype(np.float16)),
        "wod": np.ascontiguousarray(
            np.asarray(w_o, f).reshape(H, CH, C).transpose(1, 0, 2).astype(np.float16)
        ),
        "wpack": np.zeros((2, 128, 5 * C), np.float16),
        "bgt": np.ascontiguousarray(bg2),
        "emx": np.ascontiguousarray(
            np.exp(mask_bias.reshape(KC, 128).T.astype(np.float64)).astype(f)
        ),
        "ident": np.eye(128, dtype=np.float16),
    }
    # wpack columns: [wq | wk | qxT | wv | wg]
    w16 = [(0, wq16)] + [
        (wi, np.asarray(w, np.float16))
        for wi, w in ((1, w_k), (3, w_v), (4, w_g))
    ]
    for st in range(2):
        for wi, warr in w16:
            shared["wpack"][st, :, C * wi : C * (wi + 1)] = warr[128 * st : 128 * (st + 1), :]
    in_maps = []
    for i in range(NCORES):
        sl = slice(QS * i, QS * (i + 1))
        qxT16 = np.ascontiguousarray(q_x[0, sl, :].T.astype(np.float16))
        wp = shared["wpack"].copy()
        for st in range(2):
            wp[st, :, 2 * C : 2 * C + QS] = qxT16[128 * st : 128 * (st + 1), :]
        in_maps.append(
            dict(
                shared,
                wpack=wp,
                pairT=np.ascontiguousarray(
                    pair_bias[0, :, sl, :]
                    .transpose(0, 2, 1)
                    .astype(np.float16)
                    .reshape(H, KC, 128, QS)
                    .transpose(0, 2, 1, 3)
                ),
            )
        )
    return in_maps


def kernel(
    q_x, kv_x, pair_bias, mask_bias, w_q, w_k, w_v, w_g, b_g, w_o, b_o, **run_kwargs
):
    nc = get_nc()
    in_maps = make_in_maps(
        q_x, kv_x, pair_bias, mask_bias, w_q, w_k, w_v, w_g, b_g, w_o
    )
    res = run_bass_kernel_spmd(nc, in_maps, core_ids=list(range(NCORES)), **run_kwargs)
    parts = []
    for i in range(NCORES):
        # y4 arrives [pair, p, (hh, a*256+c)]; head h = 4t+2p+hh, q = a*128+p
        y4 = res.results[i]["y4"].reshape(4, 128, 2, 2, 256).astype(np.float32)
        dn = res.results[i]["den"]  # [H, QS] softmax denominators
        acc = np.zeros((QS, C), np.float32)
        for pr in range(4):
            for hh in range(2):
                h = 4 * (pr // 2) + 2 * (pr % 2) + hh
                yh = y4[:, :, hh][pr].transpose(1, 0, 2).reshape(QS, C)
                acc += yh / dn[h][:, None]
        parts.append(acc)
    out = np.concatenate(parts, axis=0) + np.asarray(b_o, np.float32)[None, :]
    kernel.last_result = res
    return out[None].astype(np.float32)


# revision 26
# speedup vs baseline: 1.0872x; 1.0872x over previous
"""Bias-augmented attention (AlphaFold-style) on 8 Trainium2 NeuronCores.

Problem: B=1, Q=K=2048, C_IN=256, H=8, CH=32
    q = (q_x @ w_q) / sqrt(CH); k = kv_x @ w_k; v = kv_x @ w_v   (per head)
    a = softmax(q k^T + pair_bias + mask_bias)
    o = (a v) * sigmoid(q_x @ w_g + b_g)
    out = o @ w_o + b_o

Sharding: data-parallel over query rows. Core i handles q rows
[256*i, 256*(i+1)), all 8 heads.

v2 layout (changes vs the 84us baseline, driven by the NTFF trace):
  * pair_bias DMA'd in 32 big transfers (2KB contiguous per partition,
    256KB each) instead of 64 small ones: the Sync engine's ~600ns
    serial DMA_DIRECT2D issue rate was the main-loop bottleneck.
  * gate sigmoids become tanh (same ACT table set as exp) packed into
    128 partitions: sigmoid(z) = 0.5*tanh(z/2)+0.5. One table set for
    the whole kernel; a dummy exp at t=0 preloads it off the critical
    path. Gate matmuls use col-group tile_position to pack 4 heads
    onto the 128 PSUM partitions; two tanh ACTIVATEs cover all heads.
  * w_o, gate, and output-projection path in fp16 (fp32r matmuls ran
    in fp32-HIGH mode at ~410ns vs ~110ns fp16).
  * y exported per head-pair as fp16 (4 DMAs instead of 8 fp32 ones).
  * scores S^T computed transposed with a ones-augmented V (M=33) so
    one accumulating matmul chain gives numerator and denominator;
    mask folds in as exp(mask) row scaling of V-hat; exp runs with a
    -3 bias (cancels on the host); normalization happens on the host.
  * pair folds into scores via fp16 identity-matmul PSUM accumulation;
    A@V alternates even/odd-chunk accumulators in different PSUM banks
    and PE column groups; emission software-pipelines QK ahead of A@V.
"""

import math
import sys

for _p in ("/opt/trn_rl_repo",):
    if _p not in sys.path:
        sys.path.insert(0, _p)

import numpy as np

import concourse.bass as bass
import concourse.mybir as mybir
import concourse.tile as tile
from concourse import bacc
from concourse.bass_utils import run_bass_kernel_spmd

F32 = mybir.dt.float32
F16 = mybir.dt.float16

B, Q, K, C, H, CH = 1, 2048, 2048, 256, 8, 32
NCORES = 8
QS = Q // NCORES  # 256 query rows per core
KC = K // 128  # 16 key chunks of 128


def build_nc():
    nc = bacc.Bacc("TRN2", target_bir_lowering=False, debug=False)

    # ---- DRAM I/O (per-core shard shapes) ----
    # pairT [h][p][kc][q]: per-partition kc-contiguous (8KB runs per head)
    pairT = nc.dram_tensor("pairT", [H, 128, KC, QS], F16, kind="ExternalInput").ap()
    wpack = nc.dram_tensor("wpack", [2, 128, 5 * C], F16, kind="ExternalInput").ap()
    kvxT = nc.dram_tensor("kvxT", [C, K], F16, kind="ExternalInput").ap()
    # w_o as [d=CH, h, c] so each head's slice sits at partition base 0
    wod = nc.dram_tensor("wod", [CH, H, C], F16, kind="ExternalInput").ap()
    # b_g/2 packed [32*(h%4)+d, h//4]
    bgt = nc.dram_tensor("bgt", [128, 2], F32, kind="ExternalInput").ap()
    emx = nc.dram_tensor("emx", [128, KC], F32, kind="ExternalInput").ap()
    ident_d = nc.dram_tensor("ident", [128, 128], F16, kind="ExternalInput").ap()
    y4 = nc.dram_tensor("y4", [4, 128, 2 * 512], F16, kind="ExternalOutput").ap()
    den = nc.dram_tensor("den", [H, QS], F32, kind="ExternalOutput").ap()

    with tile.TileContext(nc) as tc:
        with (
            tc.tile_pool(name="const", bufs=1) as const_pool,
            tc.tile_pool(name="pt", bufs=10) as pt_pool,
            tc.tile_pool(name="exps", bufs=6) as exp_pool,
            tc.tile_pool(name="head", bufs=3) as head_pool,
            tc.tile_pool(name="mm", bufs=2, space="PSUM") as mmsum,
            tc.tile_pool(name="otsum", bufs=1, space="PSUM") as otsum_pool,
            tc.tile_pool(name="yp", bufs=2, space="PSUM") as ypool,
        ):
            # exp bias (-3): keeps E inside fp16 range; cancels on the host
            negc = const_pool.tile([128, 1], F32, tag="negc")
            nc.vector.memset(negc, -3.0)

            # ---- lead-in DMAs, split across both HWDGE rings (Sync+Scalar)
            # plus SWDGE (GpSimd) for small constants: each ring issues one
            # 128-partition DMA per ~650ns, so a single ring serializes the
            # whole lead-in.
            pt_tiles = {}

            def dma_pt(h, j, eng=None):
                key = (h, j)
                pt_tiles[key] = pt_pool.tile([128, 8, QS], F16, tag="pt", name="pt")
                (eng or nc.sync).dma_start(
                    out=pt_tiles[key],
                    in_=pairT[h, :, 8 * j : 8 * j + 8, :],
                )

            # wpack columns: [wq | wk | qxT | wv | wg] so the first-768-col
            # "front" DMA carries everything qT/kT00 needs
            wpk = [
                const_pool.tile([128, 5 * C], F16, tag=f"wpk{s}", name=f"wpk{s}")
                for s in range(2)
            ]
            wq_s = [wpk[s][:, 0:C] for s in range(2)]
            wk_s = [wpk[s][:, C : 2 * C] for s in range(2)]
            qxT_s = [wpk[s][:, 2 * C : 2 * C + QS] for s in range(2)]
            wv_s = [wpk[s][:, 3 * C : 4 * C] for s in range(2)]
            wg_s = [wpk[s][:, 4 * C : 5 * C] for s in range(2)]
            kvxT_s = [
                const_pool.tile([128, K], F16, tag=f"kvxT{st}", name=f"kvxT{st}")
                for st in range(2)
            ]

            def dma_wpk(s, lo, hi, eng):
                eng.dma_start(out=wpk[s][:, lo:hi], in_=wpack[s, :, lo:hi])

            def dma_kvx(st, lo, hi, eng):
                eng.dma_start(
                    out=kvxT_s[st][:, lo:hi],
                    in_=kvxT[128 * st : 128 * (st + 1), lo:hi],
                )

            # Scalar ring: weights first, then the step-0 pair half, then the
            # dummy exp (nothing may queue after it or exp0 is FIFO-blocked)
            dma_wpk(1, 0, 3 * C, nc.scalar)
            dma_kvx(1, 0, 512, nc.scalar)
            dma_wpk(1, 3 * C, 5 * C, nc.scalar)
            dma_kvx(1, 512, 2048, nc.scalar)
            dma_pt(1, 0, nc.scalar)
            # dummy exp: forces the exp_and_others ACT table load early
            scr = const_pool.tile([128, 1], F32, tag="scr")
            nc.scalar.activation(
                out=scr, in_=negc, func=mybir.ActivationFunctionType.Exp, bias=negc
            )

            # Sync ring: all weights ahead of the bulky pair halves
            ident_t = const_pool.tile([128, 128], F16, tag="ident")
            nc.sync.dma_start(out=ident_t, in_=ident_d)
            dma_wpk(0, 0, 3 * C, nc.sync)
            dma_kvx(0, 0, 512, nc.sync)
            dma_wpk(0, 3 * C, 5 * C, nc.sync)
            dma_kvx(0, 512, 2048, nc.sync)
            dma_pt(0, 0)
            dma_pt(0, 1)
            dma_pt(1, 1)
            dma_pt(2, 0)
            dma_pt(3, 0)

            # SWDGE for the small constants
            em = const_pool.tile([128, KC], F32, tag="em")
            nc.gpsimd.dma_start(out=em, in_=emx)
            bgt_sb = const_pool.tile([128, 2], F32, tag="bgt")
            nc.gpsimd.dma_start(out=bgt_sb, in_=bgt)
            wo_sb = const_pool.tile([CH, H, C], F16, tag="wo")
            nc.gpsimd.dma_start(out=wo_sb, in_=wod)
            wo_h = [wo_sb[:, h, :] for h in range(H)]

            # remaining pair transfers issued 1/step from the streaming loop
            dma_deferred = []
            for h2, j in ((2, 1), (4, 0), (4, 1), (6, 0), (6, 1)):
                dma_deferred.append((h2, j))
                dma_deferred.append((h2 + 1, j))

            # ---- projections ----
            kT = [[None] * (K // 512) for _ in range(2)]
            qT = [None, None]
            vhat = [None] * KC

            def emit_kT(t, n):
                kt_nt = const_pool.tile([128, 512], F16, tag=f"kT{t}_{n}")
                ps = ypool.tile([128, 512], F32, tag="yp", name="ps")
                for srt in range(2):
                    nc.tensor.matmul(
                        ps,
                        wk_s[srt][:, 128 * t : 128 * (t + 1)],
                        kvxT_s[srt][:, 512 * n : 512 * (n + 1)],
                        start=(srt == 0),
                        stop=(srt == 1),
                    )
                nc.vector.tensor_copy(kt_nt, ps)
                kT[t][n] = kt_nt

            def emit_qT(t):
                qT_t = const_pool.tile([128, QS], F16, tag=f"qT{t}")
                ps = ypool.tile([128, 512], F32, tag="yp", name="ps")[:, 0:QS]
                for srt in range(2):
                    nc.tensor.matmul(
                        ps,
                        wq_s[srt][:, 128 * t : 128 * (t + 1)],
                        qxT_s[srt],
                        start=(srt == 0),
                        stop=(srt == 1),
                    )
                nc.vector.tensor_copy(qT_t, ps)
                qT[t] = qT_t

            def emit_vhat(c):
                # vhat[c][p, h, 0:32] = V[128c+p, 32h+d] * exp(mask)[128c+p]
                # vhat[c][p, h, 32]   = exp(mask)[128c+p]
                vh = const_pool.tile([128, H, CH + 1], F16, tag=f"vhat{c}")
                ps = ypool.tile([128, 512], F32, tag="yp", name="ps")[:, 0:C]
                for srt in range(2):
                    nc.tensor.matmul(
                        ps,
                        kvxT_s[srt][:, 128 * c : 128 * (c + 1)],
                        wv_s[srt],
                        start=(srt == 0),
                        stop=(srt == 1),
                    )
                emc = em[:, c : c + 1]
                nc.vector.tensor_scalar_mul(
                    vh[:, :, 0:CH], ps.rearrange("p (h d) -> p h d", h=H), emc
                )
                nc.vector.tensor_copy(vh[:, :, CH : CH + 1], emc.broadcast_to((128, H, 1)))
                vhat[c] = vh

            # ---- gates: g = sigmoid(qx@wg + b_g) = 0.5*tanh((qx@wg)/2 + b_g/2)+0.5
            # (tanh shares the exp ACT table set: no table switch mid-stream).
            # Heads packed on partitions: head h=(4t+s) at rows 32s via
            # col-group tile_position, psum col block 512t so consecutive
            # chains drain alternating banks.
            gate_state = {}
            gth = const_pool.tile([128, 2, QS], F16, tag="gth")
            gta = const_pool.tile([128, 2, QS], F16, tag="gta")
            gT = [None] * H

            def emit_gate_mm(shalf):
                if "gps0" not in gate_state:
                    gate_state["gps0"] = ypool.tile([128, 512], F32, tag="yp", name="g0")
                    gate_state["gps1"] = ypool.tile([128, 512], F32, tag="yp", name="g1")
                for s in (2 * shalf, 2 * shalf + 1):
                    for t in range(2):
                        gps = gate_state[f"gps{t}"]
                        h = 4 * t + s
                        for srt in range(2):
                            nc.tensor.matmul(
                                gps[32 * s : 32 * s + 32, 0:QS],
                                wg_s[srt][:, CH * h : CH * (h + 1)],
                                qxT_s[srt],
                                start=(srt == 0),
                                stop=(srt == 1),
                                tile_position=(0, 32 * s),
                                skip_group_check=True,
                            )

            def emit_gate_act():
                for t in range(2):
                    nc.scalar.activation(
                        out=gth[:, t, :],
                        in_=gate_state[f"gps{t}"][:, 0:QS],
                        func=mybir.ActivationFunctionType.Tanh,
                        scale=0.5,
                        bias=bgt_sb[:, t : t + 1],
                    )
                nc.vector.tensor_scalar(
                    gta, gth, 0.5, 0.5, mybir.AluOpType.mult, mybir.AluOpType.add
                )
                for h in range(H):
                    t, s = h // 4, h % 4
                    g_t = const_pool.tile([CH, QS], F16, tag=f"gT{h}")
                    nc.vector.tensor_copy(g_t, gta[32 * s : 32 * s + 32, t, :])
                    gT[h] = g_t

            emit_qT(0)
            emit_kT(0, 0)

            # per-step deferred work; inner lists are popped one list per step
            deferred = [
                [],
                [("vhat", 0), ("vhat", 1), ("kT", (0, 1))],
                [("vhat", 2), ("vhat", 3)],
                [("kT", (0, 2)), ("vhat", 4), ("vhat", 5)],
                [("kT", (0, 3)), ("vhat", 6), ("vhat", 7)],
                [("kT", (1, 0)), ("vhat", 8), ("vhat", 9)],
                [("kT", (1, 1)), ("vhat", 10), ("vhat", 11)],
                [("kT", (1, 2)), ("vhat", 12), ("vhat", 13)],
                [("kT", (1, 3)), ("vhat", 14), ("vhat", 15)],
                [("qT", None), ("gmm", 0), ("gmm", 1)],
                [("gact", None)],
            ]

            # denominators for all heads, exported once at the end
            den_sb = const_pool.tile([CH + 1, H * QS], F32, tag="den")

            # ---- streaming attention, software-pipelined ----
            # Steps iterate over head PAIRS x chunk-pairs: the two heads of
            # a pair live on adjacent kT/qT row-strips, so their QK matmuls
            # run concurrently on different PE row-groups AND drain into
            # different PSUM banks (same-bank concurrent drains are fatal).
            steps = [(t, p, cg) for t in range(2) for p in range(2) for cg in range(KC // 2)]
            tail_queue = []
            ot_by_pair = {}
            pair_state = {}

            def emit_qk(i):
                t, p, cg = steps[i]
                hA, hB = 4 * t + 2 * p, 4 * t + 2 * p + 1
                c0, c1 = 2 * cg, 2 * cg + 1
                j, m = cg // 4, cg % 4
                ptA = pt_tiles[(hA, j)][:, 2 * m : 2 * m + 2, :]
                ptB = pt_tiles[(hB, j)][:, 2 * m : 2 * m + 2, :]
                # sp halves by head: [hA-c0 | hA-c1 | hB-c0 | hB-c1]
                sp = mmsum.tile([128, 4 * QS], F32, tag="sp", name="sp")
                # issue order alternates banks: hA-c0 (a), hB-c0 (b), hA-c1
                # (a), hB-c1 (b) -> concurrent row-strip pairs never share a
                # draining bank
                for q, (hh, cc) in enumerate(
                    [(2 * p, c0), (2 * p + 1, c0), (2 * p, c1), (2 * p + 1, c1)]
                ):
                    quarter = [0, 2, 1, 3][q]
                    nc.tensor.matmul(
                        sp[:, QS * quarter : QS * (quarter + 1)],
                        kT[t][cc // 4][32 * hh : 32 * hh + 32, 128 * (cc % 4) : 128 * (cc % 4 + 1)],
                        qT[t][32 * hh : 32 * hh + 32, :],
                        start=(q < 2),
                        stop=True,
                        tile_position=(32 * hh, 0),
                        skip_group_check=True,
                    )
                # S^T += pair^T via fp16 identity-matmul accumulate on the PE
                # (half A -> bank a, half B -> bank b)
                for half, pt_half in ((0, ptA), (1, ptB)):
                    nc.tensor.matmul(
                        sp[:, 512 * half : 512 * (half + 1)],
                        ident_t,
                        pt_half.rearrange("p a q -> p (a q)"),
                        start=False,
                        stop=True,
                        skip_group_check=True,
                    )
                e_t = exp_pool.tile([128, 4 * QS], F16, tag="E", name="E")
                nc.scalar.activation(
                    out=e_t, in_=sp, func=mybir.ActivationFunctionType.Exp, bias=negc
                )
                return e_t

            def emit_av(i, e_t):
                t, p, cg = steps[i]
                hA, hB = 4 * t + 2 * p, 4 * t + 2 * p + 1
                c0, c1 = 2 * cg, 2 * cg + 1
                if cg == 0:
                    # one even + one odd accumulator per pair, two heads side
                    # by side: even chunks hit PE column-group 0, odd chunks
                    # column-group 2, in different PSUM banks
                    ot_by_pair[(t, p)] = (
                        otsum_pool.tile([CH + 1, 2 * QS], F32, tag="ote", name="ote"),
                        otsum_pool.tile([97, 2 * QS], F32, tag="oto", name="oto"),
                    )
                ote, oto = ot_by_pair[(t, p)]
                for hh, cc, quarter in (
                    (0, c0, 0),
                    (0, c1, 1),
                    (1, c0, 2),
                    (1, c1, 3),
                ):
                    out, row = (ote, 0) if cc % 2 == 0 else (oto, 64)
                    nc.tensor.matmul(
                        out[row : row + CH + 1, QS * hh : QS * (hh + 1)],
                        vhat[cc][:, (hA, hB)[hh], :],
                        e_t[:, QS * quarter : QS * (quarter + 1)],
                        start=(cg == 0 and hh == 0),
                        stop=(cg == KC // 2 - 1),
                        tile_position=(0, row),
                        skip_group_check=True,
                    )
                if cg == KC // 2 - 1:
                    if (t, p) == (1, 1):
                        # last pair: per-head merge+proj chain (shorter drain;
                        # safe only here — no successor reuses ote/oto)
                        tail_queue.append(("mph", (t, p, 0)))
                        tail_queue.append(("mph", (t, p, 1)))
                    else:
                        tail_queue.append(("merge", (t, p)))
                        tail_queue.append(("proj", (t, p, 0)))
                        tail_queue.append(("proj", (t, p, 1)))

            def emit_tail(stage):
                kind, arg = stage
                if kind == "mph":
                    t, p, hh = arg
                    h = 4 * t + 2 * p + hh
                    ote, oto = ot_by_pair[(t, p)]
                    sl = slice(QS * hh, QS * (hh + 1))
                    ots2 = head_pool.tile([CH + 1, QS], F32, tag="ots2", name="ots2")
                    nc.vector.tensor_copy(ots2, ote[:, sl])
                    otf2 = head_pool.tile([CH + 1, QS], F32, tag="otf2", name="otf2")
                    nc.vector.tensor_add(otf2, oto[64 : 64 + CH + 1, sl], ots2)
                    nc.vector.tensor_copy(
                        den_sb[CH : CH + 1, QS * h : QS * (h + 1)],
                        otf2[CH : CH + 1, :],
                    )
                    gom = head_pool.tile([CH, QS], F16, tag="gom", name="gom")
                    with nc.allow_low_precision(reason="fp16 gate/proj path"):
                        nc.vector.tensor_mul(gom, otf2[0:CH, :], gT[h])
                    y_ps = ypool.tile([128, 512], F32, tag="yp", name="yps")
                    for qc in range(QS // 128):
                        nc.tensor.matmul(
                            y_ps[:, 256 * qc : 256 * (qc + 1)],
                            gom[:, 128 * qc : 128 * (qc + 1)],
                            wo_h[h],
                            start=(qc == 0),
                            stop=True,
                            skip_group_check=True,
                        )
                    ysbh = head_pool.tile([128, 512], F16, tag="ysbh", name="ysbh")
                    nc.vector.tensor_copy(ysbh, y_ps)
                    nc.sync.dma_start(
                        out=y4[2 * t + p, :, 512 * hh : 512 * (hh + 1)], in_=ysbh
                    )
                elif kind == "merge":
                    t, p = arg
                    ote, oto = ot_by_pair[(t, p)]
                    # merge even/odd accumulators for both heads at once; the
                    # add reads PSUM at base 64 plus SBUF at base 0 (legal:
                    # only SB+SB bases must match; max one PSUM input)
                    ots = head_pool.tile([CH + 1, 2 * QS], F32, tag="ots", name="ots")
                    nc.vector.tensor_copy(ots, ote)
                    otf = head_pool.tile([CH + 1, 2 * QS], F32, tag="otf", name="otf")
                    nc.vector.tensor_add(otf, oto[64 : 64 + CH + 1, :], ots)
                    hA = 4 * t + 2 * p
                    nc.vector.tensor_copy(
                        den_sb[CH : CH + 1, QS * hA : QS * (hA + 2)],
                        otf[CH : CH + 1, :],
                    )
                    pair_state[(t, p)] = otf
                else:
                    t, p, hh = arg
                    h = 4 * t + 2 * p + hh
                    otf = pair_state[(t, p)]
                    if hh == 0:
                        pair_state[(t, p, "ysb")] = head_pool.tile(
                            [128, 2, 512], F16, tag="ysb", name="ysb"
                        )
                    ysb = pair_state[(t, p, "ysb")]
                    gom = head_pool.tile([CH, QS], F16, tag="gom", name="gom")
                    with nc.allow_low_precision(reason="fp16 gate/proj path"):
                        nc.vector.tensor_mul(
                            gom, otf[0:CH, QS * hh : QS * (hh + 1)], gT[h]
                        )
                    y_ps = ypool.tile([128, 512], F32, tag="yp", name="yps")
                    for qc in range(QS // 128):
                        nc.tensor.matmul(
                            y_ps[:, 256 * qc : 256 * (qc + 1)],
                            gom[:, 128 * qc : 128 * (qc + 1)],
                            wo_h[h],
                            start=(qc == 0),
                            stop=True,
                            skip_group_check=True,
                        )
                    nc.vector.tensor_copy(ysb[:, hh, :], y_ps)
                    if hh == 1:
                        pr = 2 * t + p
                        nc.sync.dma_start(
                            out=y4[pr], in_=ysb.rearrange("p a c -> p (a c)")
                        )

            pending = []
            for i in range(len(steps)):
                e_t = emit_qk(i)
                pending.append((i, e_t))
                if len(pending) > 2:
                    emit_av(*pending.pop(0))
                if deferred:
                    for kind, arg in deferred.pop(0):
                        if kind == "vhat":
                            emit_vhat(arg)
                        elif kind == "kT":
                            emit_kT(*arg)
                        elif kind == "gmm":
                            emit_gate_mm(arg)
                        elif kind == "gact":
                            emit_gate_act()
                        else:
                            emit_qT(1)
                if dma_deferred:
                    dma_pt(*dma_deferred.pop(0))
                if tail_queue:
                    emit_tail(tail_queue.pop(0))
            while pending:
                emit_av(*pending.pop(0))
                if tail_queue:
                    emit_tail(tail_queue.pop(0))
            while tail_queue:
                emit_tail(tail_queue.pop(0))

            # ---- export denominators ----
            nc.sync.dma_start(
                out=den.rearrange("h q -> (h q)"), in_=den_sb[CH : CH + 1, :]
            )

    nc.compile()
    return nc


_NC_CACHE = None


def get_nc():
    global _NC_CACHE
    if _NC_CACHE is None:
        _NC_CACHE = build_nc()
    return _NC_CACHE


def make_in_maps(q_x, kv_x, pair_bias, mask_bias, w_q, w_k, w_v, w_g, b_g, w_o):
    f = np.float32
    q_x = np.asarray(q_x, f)
    kv_x = np.asarray(kv_x, f)
    pair_bias = np.asarray(pair_bias, f)
    mask_bias = np.asarray(mask_bias, f)
    wq16 = (np.asarray(w_q, f) / math.sqrt(CH)).astype(np.float16)
    # b_g/2 packed [32*(h%4)+d, h//4] to match the gate partition layout
    bg2 = (np.asarray(b_g, f) / 2.0).reshape(2, 4, CH).transpose(1, 2, 0).reshape(128, 2)
    shared = {
        "kvxT": np.ascontiguousarray(kv_x[0].T.astype(np.float16)),
        "wod": np.ascontiguousarray(
            np.asarray(w_o, f).reshape(H, CH, C).transpose(1, 0, 2).astype(np.float16)
        ),
        "wpack": np.zeros((2, 128, 5 * C), np.float16),
        "bgt": np.ascontiguousarray(bg2),
        "emx": np.ascontiguousarray(
            np.exp(mask_bias.reshape(KC, 128).T.astype(np.float64)).astype(f)
        ),
        "ident": np.eye(128, dtype=np.float16),
    }
    # wpack columns: [wq | wk | qxT | wv | wg]
    w16 = [(0, wq16)] + [
        (wi, np.asarray(w, np.float16))
        for wi, w in ((1, w_k), (3, w_v), (4, w_g))
    ]
    for st in range(2):
        for wi, warr in w16:
            shared["wpack"][st, :, C * wi : C * (wi + 1)] = warr[128 * st : 128 * (st + 1), :]
    in_maps = []
    for i in range(NCORES):
        sl = slice(QS * i, QS * (i + 1))
        qxT16 = np.ascontiguousarray(q_x[0, sl, :].T.astype(np.float16))
        wp = shared["wpack"].copy()
        for st in range(2):
            wp[st, :, 2 * C : 2 * C + QS] = qxT16[128 * st : 128 * (st + 1), :]
        in_maps.append(
            dict(
                shared,
                wpack=wp,
                pairT=np.ascontiguousarray(
                    pair_bias[0, :, sl, :]
                    .transpose(0, 2, 1)
                    .astype(np.float16)
                    .reshape(H, KC, 128, QS)
                    .transpose(0, 2, 1, 3)
                ),
            )
        )
    return in_maps


def kernel(
    q_x, kv_x, pair_bias, mask_bias, w_q, w_k, w_v, w_g, b_g, w_o, b_o, **run_kwargs
):
    nc = get_nc()
    in_maps = make_in_maps(
        q_x, kv_x, pair_bias, mask_bias, w_q, w_k, w_v, w_g, b_g, w_o
    )
    res = run_bass_kernel_spmd(nc, in_maps, core_ids=list(range(NCORES)), **run_kwargs)
    parts = []
    for i in range(NCORES):
        # y4 arrives [pair, p, (hh, a*256+c)]; head h = 4t+2p+hh, q = a*128+p
        y4 = res.results[i]["y4"].reshape(4, 128, 2, 2, 256).astype(np.float32)
        dn = res.results[i]["den"]  # [H, QS] softmax denominators
        acc = np.zeros((QS, C), np.float32)
        for pr in range(4):
            for hh in range(2):
                h = 4 * (pr // 2) + 2 * (pr % 2) + hh
                yh = y4[:, :, hh][pr].transpose(1, 0, 2).reshape(QS, C)
                acc += yh / dn[h][:, None]
        parts.append(acc)
    out = np.concatenate(parts, axis=0) + np.asarray(b_o, np.float32)[None, :]
    kernel.last_result = res
    return out[None].astype(np.float32)
